# revision 1
# baseline (speedup 1.0000x reference)
# Sliding-window causal multi-head attention with RoPE for Trainium2.
#
# Problem: B=4, T=2048, D=1024, H=16 heads, d_k=64, window=512.
#   q,k,v = x @ W{q,k,v}^T (split heads), RoPE(q,k), scores = q k^T / 8 with
#   mask 0 <= i-j <= 512, softmax, out = (attn @ v) concat-heads @ Wo^T.
#
# Sharding: 8 cores = (batch b in 0..3) x (sequence half). Each core computes
# output rows [half*1024, half*1024+1024) of batch b. It needs K/V for global
# rows [qbase-512, qbase+1024); for half 0 the first 512 rows don't exist and
# are zero-padded (host side), with a sums-correction term subtracted on-chip
# (padded keys contribute exp(0)=1 to the softmax denominator).
#
# On-chip pipeline (all matmuls bf16 with fp32 PSUM accumulation):
#   - x and W are cast f32->bf16 by SWDGE DMA into DRAM scratch, then
#     DMA-xbar-transposed into SBUF as x^T [m, t] and W^T [m, n] tiles.
#   - Q^T/K^T projections produce [128 = 2 heads x (evens|odds), t] tiles in
#     PSUM; RoPE is applied with host-provided cos/sin tables; the rotate-half
#     "swap" is a PE matmul with a permutation matrix.
#   - scores are computed transposed, S^T[k, q] = K Q^T, per (head, kv-block)
#     with the 5-block sliding window span; exp on ACT (scale=1/8 folded in);
#     boundary masks applied multiplicatively post-exp (gpsimd).
#   - PV uses lhsT = [V_h | ones] so the PSUM result holds both O^T (64 rows)
#     and the softmax denominator replicated (64 rows); normalization is
#     reciprocal_approx_fast + multiply, writing attnT [m', q] bf16 tiles that
#     feed the final Wo matmul directly.

import dataclasses
from contextlib import ExitStack

import numpy as np
import ml_dtypes

BF16 = ml_dtypes.bfloat16

B, T, D = 4, 2048, 1024
H, DK = 16, 64
WIN = 512
THETA = 10000.0
TQ, TKV = 1024, 1536
NBQ, NBKV = TQ // 128, TKV // 128  # 8, 12
NCHUNK = D // 128  # 8 contraction chunks
NPAIR = H // 2  # 8 head pairs

_CACHE = {}


def _pair_cols(ap2d, a, b, w):
    """From a [P, F] AP over contiguous cols, build an AP over cols
    {a..a+w} then {b..b+w} (2D free: outer count 2 step b-a)."""
    base = ap2d[:, a : a + w]
    return dataclasses.replace(base, ap=[base.ap[0], [b - a, 2], [1, w]])


def _build(debug_dumps=False, wpool_bufs=2):
    import concourse.bass as bass
    import concourse.bacc as bacc
    import concourse.mybir as mybir
    import concourse.tile as tile

    dt = mybir.dt
    F32, BF = dt.float32, dt.bfloat16
    AF = mybir.ActivationFunctionType
    OP = mybir.AluOpType

    nc = bacc.Bacc("TRN2", target_bir_lowering=False, debug=False, num_devices=8)

    # ---- DRAM I/O ----
    x_kv = nc.dram_tensor("x_kv", [TKV, D], F32, kind="ExternalInput").ap()
    w_in = {
        n: nc.dram_tensor(n, [D, D], F32, kind="ExternalInput").ap()
        for n in ("wq", "wk", "wv", "wo")
    }
    # small constants batched into two loads: pack1 = [cos|sin|pswap|masks],
    # pack2 = [corr|cselE|cselO]
    pack1_in = nc.dram_tensor("pack1", [128, 2 * TKV + 384], BF, kind="ExternalInput").ap()
    pack2_in = nc.dram_tensor("pack2", [2, TQ + 256], BF, kind="ExternalInput").ap()
    sign_in = nc.dram_tensor("sign_t", [128, 1], F32, kind="ExternalInput").ap()
    out_d = nc.dram_tensor("out", [TQ, D], F32, kind="ExternalOutput").ap()

    # DRAM scratch for the cast-then-transpose path. All four weight casts
    # share ONE scratch tensor: the WAR dependency (next cast overwrites what
    # the previous transpose read) staggers the casts so the first-needed
    # weights aren't slowed by SDMA fair-share across every queued transfer.
    x_bf = nc.dram_tensor("x_bf", [TKV, D], BF, kind="Internal").ap()
    w_bf_shared = nc.dram_tensor("w_bf", [D, D], BF, kind="Internal").ap()
    w_bf = {n: w_bf_shared for n in ("wq", "wk", "wv", "wo")}

    with ExitStack() as ctx:
        tc = ctx.enter_context(tile.TileContext(nc))

        big = ctx.enter_context(tc.tile_pool(name="big", bufs=1))
        wpool = ctx.enter_context(tc.tile_pool(name="wpool", bufs=wpool_bufs))
        ab = ctx.enter_context(tc.tile_pool(name="ab", bufs=4))
        epool = ctx.enter_context(tc.tile_pool(name="epool", bufs=13))
        rpool = ctx.enter_context(tc.tile_pool(name="rpool", bufs=2))
        stpool = ctx.enter_context(tc.tile_pool(name="stpool", bufs=2))
        # PSUM budget (8 banks): proj/swap/Wo pool 3x1 + scores 2x2 + pv 1x1
        mmps = ctx.enter_context(tc.tile_pool(name="mmps", bufs=3, space="PSUM"))
        scps = ctx.enter_context(tc.tile_pool(name="scps", bufs=2, space="PSUM"))
        pvps = ctx.enter_context(tc.tile_pool(name="pvps", bufs=1, space="PSUM"))

        # ---- persistent SBUF ----
        xT = big.tile([128, NCHUNK, TKV], BF)
        qT = big.tile([128, NPAIR, TQ], BF)
        kT = big.tile([128, NPAIR, TKV], BF)
        vS = big.tile([128, NBKV, 1024], BF)  # 16 heads x 64 cols
        onesS = big.tile([128, 64], BF)
        attnT = big.tile([128, NPAIR, TQ], BF)
        pack1S = big.tile([128, 2 * TKV + 384], BF)
        pack2S = big.tile([2, TQ + 256], BF)
        signS = big.tile([128, 1], F32)
        cosS = pack1S[:, 0:TKV]
        sinS = pack1S[:, TKV : 2 * TKV]
        pswapS = pack1S[:, 2 * TKV : 2 * TKV + 128]
        maskS = pack1S[:, 2 * TKV + 128 : 2 * TKV + 384]
        corrS = pack2S[:, 0:TQ]
        cselES = pack2S[:, TQ : TQ + 128]
        cselOS = pack2S[:, TQ + 128 : TQ + 256]

        nc.sync.dma_start(out=pack1S, in_=pack1_in)
        nc.sync.dma_start(out=pack2S, in_=pack2_in)
        nc.sync.dma_start(out=signS, in_=sign_in)
        nc.vector.memset(onesS, 1.0)

        # ---- input prep: cast f32->bf16 (DRAM->DRAM) then xbar-transpose
        # into SBUF.  Cast DMAs are staged into "waves" with explicit deps so
        # the first-needed data isn't slowed by SDMA fair-share round-robin
        # across every queued transfer.
        _weng = [nc.sync, nc.scalar]
        _wave_last = [None]

        def _stage(dma_instr):
            return dma_instr

        def _wave_done(instr):
            _wave_last[0] = instr

        def cast_x(half):
            cs = slice(half * 512, half * 512 + 512)
            c1 = _stage(nc.gpsimd.dma_start(out=x_bf[:, cs], in_=x_kv[:, cs]))
            eng = _weng[half % 2]
            eng.dma_start(
                out=xT[:, 4 * half : 4 * half + 4, :], in_=x_bf[:, cs], transpose=True
            )
            return c1

        def prep_w(name, eo_permute=False, halves=False):
            # For Wq/Wk the rows (head output dims) are permuted during the
            # cast so each head's 64 dims land as (evens, odds) — RoPE's
            # rotate-half then only needs 32-row group swaps, and the
            # projection lhsT slices stay contiguous.
            wt = wpool.tile([128, NCHUNK, D], BF, tag="wT")
            R = D
            last = None
            nhal = 2 if halves else 1
            w = D // nhal
            for hf in range(nhal):
                cs = slice(hf * w, hf * w + w)
                if eo_permute:
                    for e in range(2):
                        src = w_in[name][:, cs]
                        src = dataclasses.replace(
                            src,
                            offset=src.offset + e * R,
                            ap=[[64 * R, 16], [2 * R, 32], [1, w]],
                        )
                        dst = w_bf[name][:, cs]
                        dst = dataclasses.replace(
                            dst,
                            offset=dst.offset + 32 * e * R,
                            ap=[[64 * R, 16], [R, 32], [1, w]],
                        )
                        last = _stage(nc.gpsimd.dma_start(out=dst, in_=src))
                else:
                    last = _stage(
                        nc.gpsimd.dma_start(out=w_bf[name][:, cs], in_=w_in[name][:, cs])
                    )
                eng = _weng[0]
                _weng.append(_weng.pop(0))
                nch = NCHUNK // nhal
                eng.dma_start(
                    out=wt[:, hf * nch : hf * nch + nch, :],
                    in_=w_bf[name][:, cs],
                    transpose=True,
                )
            return wt, last

        _evac_alt = [0]

        def _evac(out, in_):
            # alternate psum evacuations between ACT and DVE queues
            _evac_alt[0] ^= 1
            if _evac_alt[0]:
                nc.scalar.copy(out=out, in_=in_)
            else:
                nc.vector.tensor_copy(out, in_)

        def proj_tile(wt, dest, r, tch, coff):
            # one roped Q^T/K^T tile: pair r, t-chunk tch
            tsl = slice(coff + tch * 512, coff + tch * 512 + 512)
            osl = slice(tch * 512, tch * 512 + 512)
            ps = mmps.tile([128, 512], F32, tag="mm")
            for c in range(NCHUNK):
                nc.tensor.matmul(
                    ps,
                    wt[:, c, r * 128 : r * 128 + 128],
                    xT[:, c, tsl],
                    start=(c == 0),
                    stop=(c == NCHUNK - 1),
                )
            # evacuate psum to bf16 once (ACT), then both RoPE muls run in
            # DVE 2x mode on all-bf16 SBUF operands
            pb = ab.tile([128, 512], BF, tag="pb")
            _evac(pb, ps)
            w1 = ab.tile([128, 512], BF, tag="w1")
            t2 = ab.tile([128, 512], BF, tag="t2")
            nc.vector.tensor_mul(w1, pb, sinS[:, tsl])
            nc.vector.tensor_mul(t2, pb, cosS[:, tsl])
            us = mmps.tile([128, 512], F32, tag="mm")
            nc.tensor.matmul(us, pswapS, w1, start=True, stop=True)
            # rope = swap(P*sin) * sign + P*cos
            nc.vector.scalar_tensor_tensor(
                out=dest[:, r, osl],
                in0=us,
                scalar=signS[:, 0:1],
                in1=t2,
                op0=OP.mult,
                op1=OP.add,
            )

        # SWDGE casts drain serially (single ring) — order by first use:
        # Wq, then x, then Wv, Wk, Wo.
        wqT, _ = prep_w("wq", eo_permute=True)
        cast_x(0)
        cast_x(1)
        wvT, _ = prep_w("wv")
        wkT, _ = prep_w("wk", eo_permute=True)
        woT, _ = prep_w("wo")

        # ---- Q projection (all pairs) interleaved with V projection so the
        # PE queue has V work to fill Q's rope-evacuation bubbles ----
        def v_tile(tt, nh):
            ps = mmps.tile([128, 512], F32, tag="mm")
            for c in range(NCHUNK):
                nc.tensor.matmul(
                    ps,
                    xT[:, c, tt * 128 : tt * 128 + 128],
                    wvT[:, c, nh * 512 : nh * 512 + 512],
                    start=(c == 0),
                    stop=(c == NCHUNK - 1),
                )
            _evac(vS[:, tt, nh * 512 : nh * 512 + 512], ps)

        vlist = [(tt, nh) for tt in range(NBKV) for nh in range(2)]  # 24
        qlist = [(r, tch) for r in range(NPAIR) for tch in range(2)]  # 16
        for rnd in range(8):  # 2 q-tiles + 3 v-tiles per round
            for j in range(2):
                r, tch = qlist[2 * rnd + j]
                proj_tile(wqT, qT, r, tch, 512)  # queries = kv rows 512..1536
            for j in range(3):
                v_tile(*vlist[3 * rnd + j])

        # ---- K projection + attention, interleaved per head-pair so the
        # PE's in-order queue always has data-ready matmuls while the
        # attention chain waits on ACT/DVE.
        # kv block b serves q blocks g in [max(0,b-4), min(b,7)]
        for p in range(NPAIR):
            for tch in range(3):
                proj_tile(wkT, kT, p, tch, 0)
            for sub in range(2):  # 0: head 2p (rows 0:64), 1: head 2p+1 (rows 64:128)
                h = 2 * p + sub
                rows = slice(64 * sub, 64 * sub + 64)
                e_tiles = {}
                for b in range(NBKV):
                    glo, ghi = max(0, b - 4), min(b, NBQ - 1)
                    span = (ghi - glo + 1) * 128
                    q0 = glo * 128
                    sc = scps.tile([128, 640], F32, tag="sc")
                    for c0 in range(0, span, 512):
                        c1 = min(c0 + 512, span)
                        nc.tensor.matmul(
                            sc[:, c0:c1],
                            kT[rows, p, b * 128 : b * 128 + 128],
                            qT[rows, p, q0 + c0 : q0 + c1],
                            start=True,
                            stop=True,
                        )
                    et = epool.tile([128, 640], BF, tag="et")
                    nc.scalar.activation(
                        out=et[:, 0:span], in_=sc[:, 0:span], func=AF.Exp, scale=0.125
                    )
                    # boundary masks (multiplicative, post-exp) on DVE
                    meng = nc.vector
                    has_diag = b >= 4  # q block g=b-4 at span cols 0:128
                    has_triu = b <= NBQ - 1  # q block g=b at last 128 cols
                    if has_diag and has_triu:
                        sel = _pair_cols(et[:, 0:640], 0, span - 128, 128)
                        meng.tensor_mul(sel, sel, maskS[:, 0:256])
                    elif has_diag:
                        meng.tensor_mul(et[:, 0:128], et[:, 0:128], maskS[:, 0:128])
                    else:
                        sl = slice(span - 128, span)
                        meng.tensor_mul(et[:, sl], et[:, sl], maskS[:, 128:256])
                    e_tiles[b] = (et, q0, span)

                # PV accumulation per 512-col q-half: a [128, 512] psum tile
                # (one bank, double-buffered). The start=True matmul of each
                # half covers the full bank extent (b=3 spans [0,512), b=8
                # spans [512,1024) exactly) before any accumulation.
                # O rows and sums rows: even heads put O low / sums high,
                # odd heads the reverse, so attnT chunk p is [head 2p; head
                # 2p+1] and every later elementwise op stays lane-aligned.
                olo, rlo = (0, 64) if sub == 0 else (64, 0)
                csel = cselES if sub == 0 else cselOS
                for qh in range(2):
                    qa0, qb0 = qh * 512, qh * 512 + 512
                    starter = 3 if qh == 0 else 8
                    order = [starter] + [
                        b
                        for b in range(NBKV)
                        if b != starter
                        and max(0, b - 4) * 128 < qb0
                        and (min(b, NBQ - 1) + 1) * 128 > qa0
                    ]
                    pv = pvps.tile([128, 512], F32, tag="pv")
                    for i, b in enumerate(order):
                        et, q0, span = e_tiles[b]
                        glo, ghi = max(0, b - 4), min(b, NBQ - 1)
                        s0 = max(glo * 128, qa0)
                        s1 = min((ghi + 1) * 128, qb0)
                        rhs = et[:, s0 - q0 : s1 - q0]
                        st = i == 0
                        nc.tensor.matmul(
                            pv[olo : olo + 64, s0 - qa0 : s1 - qa0],
                            vS[:, b, 64 * h : 64 * h + 64],
                            rhs,
                            start=st,
                            stop=False,
                            skip_group_check=True,
                            tile_position=(0, olo),
                        )
                        nc.tensor.matmul(
                            pv[rlo : rlo + 64, s0 - qa0 : s1 - qa0],
                            onesS,
                            rhs,
                            start=st,
                            stop=False,
                            skip_group_check=True,
                            tile_position=(0, rlo),
                        )
                    # softmax-denominator correction for the zero-padded keys
                    nc.tensor.matmul(
                        pv,
                        csel,
                        corrS[:, qa0:qb0],
                        start=False,
                        stop=True,
                        skip_group_check=True,
                    )
                    # normalize: attnT[rows_h] = O / sums.
                    # reciprocal_approx_fast (custom DVE op) is broken at
                    # partition base 64 on HW, so always run it at base 0;
                    # cross-partition-base operands on plain DVE ops are fine.
                    rec = rpool.tile([128, 512], F32, tag="rec")
                    lo, hi = slice(0, 64), slice(64, 128)
                    osl = attnT[
                        64 * sub : 64 * sub + 64, p, qa0:qb0
                    ]
                    if sub == 0:  # O low, sums high
                        nc.vector.tensor_copy(rec[lo, :], pv[hi, :])
                        nc.vector.reciprocal_approx_fast(out=rec[lo, :], in_=rec[lo, :])
                        nc.vector.tensor_mul(osl, pv[lo, :], rec[lo, :])
                    else:  # O high, sums low
                        nc.vector.reciprocal_approx_fast(out=rec[lo, :], in_=pv[lo, :])
                        nc.vector.tensor_mul(osl, pv[hi, :], rec[lo, :])

        if debug_dumps:
            for nm, tl, sh in (
                ("d_xT", xT, [128, NCHUNK * TKV]),
                ("d_qT", qT, [128, NPAIR * TQ]),
                ("d_kT", kT, [128, NPAIR * TKV]),
                ("d_vS", vS, [128, NBKV * 1024]),
                ("d_attnT", attnT, [128, NPAIR * TQ]),
                ("d_wqT", wqT, [128, NCHUNK * D]),
                ("d_woT", woT, [128, NCHUNK * D]),
            ):
                dd = nc.dram_tensor(nm, sh, BF, kind="ExternalOutput").ap()
                nc.sync.dma_start(out=dd, in_=tl)

        # ---- output projection ----
        for qt in range(NBQ):
            st = stpool.tile([128, D], F32, tag="st")
            for nh in range(2):
                ps = mmps.tile([128, 512], F32, tag="mm")
                for c in range(NPAIR):
                    nc.tensor.matmul(
                        ps,
                        attnT[:, c, qt * 128 : qt * 128 + 128],
                        woT[:, c, nh * 512 : nh * 512 + 512],
                        start=(c == 0),
                        stop=(c == NPAIR - 1),
                    )
                nc.scalar.copy(out=st[:, nh * 512 : nh * 512 + 512], in_=ps)
            nc.sync.dma_start(out=out_d[qt * 128 : qt * 128 + 128, :], in_=st)

    nc.compile()
    return nc


def _host_inputs(x, token_positions, Wq, Wk, Wv, Wo):
    x = np.ascontiguousarray(np.asarray(x, dtype=np.float32))
    pos = np.asarray(token_positions).astype(np.int64)
    ws = {
        "wq": np.ascontiguousarray(np.asarray(Wq, np.float32)),
        "wk": np.ascontiguousarray(np.asarray(Wk, np.float32)),
        "wv": np.ascontiguousarray(np.asarray(Wv, np.float32)),
        "wo": np.ascontiguousarray(np.asarray(Wo, np.float32)),
    }
    invf = THETA ** (-np.arange(32, dtype=np.float64) * 2.0 / DK)
    sign = np.tile(np.repeat(np.float32([-1, 1]), 32), 2).reshape(128, 1)
    perm = np.r_[32:64, 0:32, 96:128, 64:96]
    P = np.zeros((128, 128), np.float32)
    P[np.arange(128), perm] = 1.0
    pswapT = np.ascontiguousarray(P.T).astype(BF16)
    cidx = np.arange(128)[:, None]
    ridx = np.arange(128)[None, :]
    m_diag = (ridx >= cidx).astype(BF16)
    m_triu = (ridx <= cidx).astype(BF16)
    masks = np.ascontiguousarray(np.concatenate([m_diag, m_triu], axis=1))

    in_maps = []
    for core in range(8):
        b, half = divmod(core, 2)
        qbase = half * TQ
        if half == 0:
            xkv = np.concatenate([np.zeros((WIN, D), np.float32), x[b, :TQ]], axis=0)
        else:
            xkv = np.ascontiguousarray(x[b, T - TKV :])
        j = qbase - WIN + np.arange(TKV)
        jv = np.clip(j, 0, T - 1)
        posv = np.where((j >= 0) & (j < T), pos[jv], 0).astype(np.float64)
        ang = invf[:, None] * posv[None, :]  # [32, TKV]
        cos_t = np.tile(np.cos(ang), (4, 1)).astype(BF16)
        sin_t = np.tile(np.sin(ang), (4, 1)).astype(BF16)
        gi = qbase + np.arange(TQ)
        corrv = np.maximum(0, WIN - gi).astype(np.float32) if half == 0 else np.zeros(TQ, np.float32)
        corrA = np.minimum(corrv, 256.0)
        corr = np.stack([corrA, corrv - corrA]).astype(BF16)
        cselE = np.zeros((2, 128), np.float32)
        cselE[:, 64:] = -1.0
        cselO = np.zeros((2, 128), np.float32)
        cselO[:, :64] = -1.0
        pack1 = np.ascontiguousarray(
            np.concatenate([cos_t, sin_t, pswapT, masks], axis=1)
        )
        pack2 = np.ascontiguousarray(
            np.concatenate([corr, cselE.astype(BF16), cselO.astype(BF16)], axis=1)
        )
        in_maps.append(
            {"x_kv": xkv, **ws, "pack1": pack1, "pack2": pack2, "sign_t": sign}
        )
    return in_maps


def _get_nc():
    if "nc" not in _CACHE:
        _CACHE["nc"] = _build()
    return _CACHE["nc"]


def kernel(x, token_positions, Wq, Wk, Wv, Wo, _trace=False):
    from concourse.bass_utils import run_bass_kernel_spmd

    nc = _get_nc()
    in_maps = _host_inputs(x, token_positions, Wq, Wk, Wv, Wo)
    res = run_bass_kernel_spmd(nc, in_maps, core_ids=list(range(8)), trace=_trace)
    _CACHE["last_result"] = res
    out = np.zeros((B, T, D), np.float32)
    for core in range(8):
        b, half = divmod(core, 2)
        out[b, half * TQ : half * TQ + TQ] = res.results[core]["out"]
    return out



# revision 12
# speedup vs baseline: 1.5628x; 1.5628x over previous
# Sliding-window causal multi-head attention with RoPE for Trainium2.
#
# Problem: B=4, T=2048, D=1024, H=16 heads, d_k=64, window=512.
#   q,k,v = x @ W{q,k,v}^T (split heads), RoPE(q,k), scores = q k^T / 8 with
#   mask 0 <= i-j <= 512, softmax, out = (attn @ v) concat-heads @ Wo^T.
#
# Sharding: 8 cores = (batch b in 0..3) x (head-group of 8 heads). Each core
# runs the full T=2048 sequence for its 8 heads and produces a PARTIAL output
# projection (contraction over its 512 attn dims); the host sums the two
# head-group partials per batch. Head split avoids the K/V window-overlap
# recompute and the zero-pad softmax correction a sequence split needs.
#
# Host-side prep: x and all weights are cast to bf16 and pre-transposed into
# the exact SBUF layouts the PE consumes (m-major lhsT tiles), so the device
# does plain contiguous DMA loads only — no SWDGE casts, no xbar transposes.
# Wq/Wk rows are eo-permuted per head so RoPE's rotate-half is a 32-row group
# swap (PE permutation matmul), as in cs336 rope with (evens|odds) packing.
#
# On-chip pipeline (all matmuls bf16 with fp32 PSUM accumulation):
#   - Q^T/K^T projections produce [128 = 2 heads x (evens|odds), t] tiles;
#     RoPE via host cos/sin tables + pswap permutation matmul.
#   - scores are computed transposed, S^T[kv, q] = K Q^T, per (head, kv
#     block) over the 5-block sliding window span; exp on ACT (scale=1/8
#     folded in); boundary masks applied multiplicatively post-exp on
#     gpsimd (otherwise idle).
#   - PV uses a two-segment lhsT AP [ones | V_h] so one matmul yields the
#     softmax denominator (rows 0:64) AND O^T (rows 64:128); normalization
#     is reciprocal_approx_fast + multiply into bf16 attnT tiles.
#   - scores of sub-step s are software-pipelined against PV of s-1 and the
#     next pair's K projection so the in-order PE queue never starves while
#     ACT drains the exp chain.

import dataclasses
from contextlib import ExitStack

import numpy as np
import ml_dtypes

BF16 = ml_dtypes.bfloat16

B, T, D = 4, 2048, 1024
H, DK = 16, 64
WIN = 512
THETA = 10000.0
NBT = T // 128  # 16 t/kv blocks
NCH = D // 128  # 8 contraction chunks
NPAIR = 4  # head pairs per core

_CACHE = {}


def _pair_cols(ap2d, a, b, w):
    """From a [P, F] AP over contiguous cols, build an AP over cols
    {a..a+w} then {b..b+w} (2D free: outer count 2 step b-a)."""
    base = ap2d[:, a : a + w]
    return dataclasses.replace(base, ap=[base.ap[0], [b - a, 2], [1, w]])


def _build(debug_dumps=False):
    import concourse.bass as bass
    import concourse.bacc as bacc
    import concourse.mybir as mybir
    import concourse.tile as tile

    dt = mybir.dt
    F32, BF = dt.float32, dt.bfloat16
    AF = mybir.ActivationFunctionType
    OP = mybir.AluOpType

    nc = bacc.Bacc("TRN2", target_bir_lowering=False, debug=False, num_devices=8)

    # ---- DRAM I/O (all device inputs are host-prepped bf16 layouts) ----
    xt_in = nc.dram_tensor("xt", [128, 4 * NCH * 512], BF, kind="ExternalInput").ap()
    wq_in = nc.dram_tensor("wq", [128, NCH * 512], BF, kind="ExternalInput").ap()
    wk_in = nc.dram_tensor("wk", [128, NCH * 512], BF, kind="ExternalInput").ap()
    wv_in = nc.dram_tensor("wv", [128, NCH * 512], BF, kind="ExternalInput").ap()
    wo_in = nc.dram_tensor("wo", [128, 4 * 1024], BF, kind="ExternalInput").ap()
    # pack = [cos (2048) | sin (2048) | pswap (128) | masks (256)]
    pack_in = nc.dram_tensor("pack", [128, 2 * T + 384], BF, kind="ExternalInput").ap()
    sign_in = nc.dram_tensor("sign_t", [128, 1], F32, kind="ExternalInput").ap()
    out_d = nc.dram_tensor("out", [T, D], F32, kind="ExternalOutput").ap()

    with ExitStack() as ctx:
        tc = ctx.enter_context(tile.TileContext(nc))

        big = ctx.enter_context(tc.tile_pool(name="big", bufs=1))
        ab = ctx.enter_context(tc.tile_pool(name="ab", bufs=4))
        epool = ctx.enter_context(tc.tile_pool(name="epool", bufs=24))
        rpool = ctx.enter_context(tc.tile_pool(name="rpool", bufs=2))
        stpool = ctx.enter_context(tc.tile_pool(name="stpool", bufs=2))
        # PSUM (8 banks): proj/swap/Wo 2x1 + scores 2x2 + pv 2x1
        mmps = ctx.enter_context(tc.tile_pool(name="mmps", bufs=2, space="PSUM"))
        scps = ctx.enter_context(tc.tile_pool(name="scps", bufs=2, space="PSUM"))
        pvps = ctx.enter_context(tc.tile_pool(name="pvps", bufs=2, space="PSUM"))

        # ---- persistent SBUF ----
        xT = big.tile([128, 4, NCH, 512], BF)  # [m-part, tch, chunk, t]
        qT = big.tile([128, NPAIR, T], BF)
        kT = big.tile([128, NPAIR, T], BF)
        # per (kv block, head): [ones(64) | V_h(64)] so one PV matmul yields
        # the softmax denominator (out rows 0:64) and O^T (rows 64:128)
        vOnes = big.tile([128, NBT, 8, 128], BF)
        attnT = big.tile([128, NPAIR, T], BF)
        wqS = big.tile([128, NCH, 512], BF)
        wkS = big.tile([128, NCH, 512], BF)
        wvS = big.tile([128, NCH, 512], BF)
        woS = big.tile([128, 4, 1024], BF)
        packS = big.tile([128, 2 * T + 384], BF)
        signS = big.tile([128, 1], F32)
        cosS = packS[:, 0:T]
        sinS = packS[:, T : 2 * T]
        pswapS = packS[:, 2 * T : 2 * T + 128]
        maskS = packS[:, 2 * T + 128 : 2 * T + 384]

        # ---- input DMAs, balanced across the two HWDGE queues and ordered
        # by first use (xt tch slices split in half across both queues) ----
        nc.sync.dma_start(out=packS, in_=pack_in)
        nc.scalar.dma_start(out=wqS, in_=wq_in)
        nc.scalar.dma_start(out=signS, in_=sign_in)
        for tch in range(4):
            c0 = tch * NCH * 512
            nc.scalar.dma_start(
                out=xT[:, tch, 0:4, :], in_=xt_in[:, c0 : c0 + 2048]
            )
            nc.sync.dma_start(
                out=xT[:, tch, 4:8, :], in_=xt_in[:, c0 + 2048 : c0 + 4096]
            )
            if tch == 0:
                nc.sync.dma_start(out=wvS, in_=wv_in)
            elif tch == 1:
                nc.scalar.dma_start(out=wkS, in_=wk_in)
            elif tch == 2:
                nc.sync.dma_start(out=woS, in_=wo_in)
        nc.vector.memset(vOnes[:, :, :, 0:64], 1.0)

        _evac_alt = [0]

        def _evac(out, in_):
            # alternate psum evacuations between ACT and DVE queues
            _evac_alt[0] ^= 1
            if _evac_alt[0]:
                nc.scalar.copy(out=out, in_=in_)
            else:
                nc.vector.tensor_copy(out, in_)

        # ---- projection tile helpers (split so the swap matmul can be
        # queued late, after other PE work, hiding the rope DVE latency) ----
        def proj_mm_r(wS, r, tch):
            ps = mmps.tile([128, 512], F32, tag="mm")
            for c in range(NCH):
                nc.tensor.matmul(
                    ps,
                    wS[:, c, r * 128 : r * 128 + 128],
                    xT[:, tch, c, :],
                    start=(c == 0),
                    stop=(c == NCH - 1),
                )
            return ps

        def rope_pre(ps, tch, evac_eng=None):
            tsl = slice(tch * 512, tch * 512 + 512)
            pb = ab.tile([128, 512], BF, tag="pb")
            if evac_eng is None:
                _evac(pb, ps)
            elif evac_eng == "v":
                nc.vector.tensor_copy(pb, ps)
            w1 = ab.tile([128, 512], BF, tag="w1")
            t2 = ab.tile([128, 512], BF, tag="t2")
            nc.vector.tensor_mul(w1, pb, sinS[:, tsl])
            nc.vector.tensor_mul(t2, pb, cosS[:, tsl])
            return w1, t2

        def rope_swap(w1, t2, dest, r, tch):
            tsl = slice(tch * 512, tch * 512 + 512)
            us = mmps.tile([128, 512], F32, tag="mm")
            nc.tensor.matmul(us, pswapS, w1, start=True, stop=True)
            # rope = swap(P*sin) * sign + P*cos
            nc.vector.scalar_tensor_tensor(
                out=dest[:, r, tsl],
                in0=us,
                scalar=signS[:, 0:1],
                in1=t2,
                op0=OP.mult,
                op1=OP.add,
            )

        def v_tile(tt):
            tch, off = tt // 4, (tt % 4) * 128
            ps = mmps.tile([128, 512], F32, tag="mm")
            for c in range(NCH):
                nc.tensor.matmul(
                    ps,
                    xT[:, tch, c, off : off + 128],
                    wvS[:, c, :],
                    start=(c == 0),
                    stop=(c == NCH - 1),
                )
            _evac(vOnes[:, tt, :, 64:128], ps)

        # ---- phase 2: Q projection interleaved with V so the PE queue has
        # V work to fill Q's rope bubbles ----
        qlist = [(r, tch) for tch in range(4) for r in range(NPAIR)]  # 16
        for i in range(16):
            r, tch = qlist[i]
            ps = proj_mm_r(wqS, r, tch)
            w1, t2 = rope_pre(ps, tch)
            v_tile(i)
            rope_swap(w1, t2, qT, r, tch)

        # ---- phase 3: K projection + attention, software-pipelined ----
        # sub-step si = 2p + sub. Scores of si interleave (in the PE queue)
        # with PV groups of si-1 and the next pair's K-projection chunks so
        # the PE keeps busy while ACT drains the per-block exp chain.
        e_tiles = {}

        def k_chunks(p):
            # 8 thunks: 4 mm chains and 4 swap finishes, swap_i after mm_i
            thunks = []
            pend = {}

            def mk_mm(tch):
                def f():
                    ps = proj_mm_r(wkS, p, tch)
                    pend[tch] = rope_pre(ps, tch, evac_eng="v")

                return f

            def mk_swap(tch):
                def f():
                    w1, t2 = pend.pop(tch)
                    rope_swap(w1, t2, kT, p, tch)

                return f

            order = [mk_mm(0), mk_mm(1), mk_swap(0), mk_mm(2), mk_swap(1),
                     mk_mm(3), mk_swap(2), mk_swap(3)]
            return order

        def sc_chunks(p, sub, si):
            rows = slice(64 * sub, 64 * sub + 64)

            def mk(b):
                def f():
                    ghi = min(b + 4, NBT - 1)
                    span = (ghi - b + 1) * 128
                    q0 = b * 128
                    sc = scps.tile([128, 640], F32, tag="sc")
                    for c0 in range(0, span, 512):
                        c1 = min(c0 + 512, span)
                        nc.tensor.matmul(
                            sc[:, c0:c1],
                            kT[rows, p, b * 128 : b * 128 + 128],
                            qT[rows, p, q0 + c0 : q0 + c1],
                            start=True,
                            stop=True,
                        )
                    et = epool.tile([128, 640], BF, tag="et")
                    nc.scalar.activation(
                        out=et[:, 0:span], in_=sc[:, 0:span], func=AF.Exp, scale=0.125
                    )
                    # boundary masks (multiplicative, post-exp) on gpsimd:
                    # causal at cols 0:128 (g=b), window at span-128 (g=b+4)
                    if b <= NBT - 5:
                        sel = _pair_cols(et[:, 0:640], 0, span - 128, 128)
                        nc.gpsimd.tensor_mul(sel, sel, maskS[:, 0:256])
                    else:
                        nc.gpsimd.tensor_mul(
                            et[:, 0:128], et[:, 0:128], maskS[:, 0:128]
                        )
                    e_tiles[(si, b)] = (et, q0, span)

                return f

            return [mk(b) for b in range(NBT)]

        def pv_chunks(p, sub, si):
            h = 2 * p + sub
            rows_h = slice(64 * sub, 64 * sub + 64)

            def mk(qh):
                def f():
                    qa0, qb0 = qh * 512, qh * 512 + 512
                    bstar = max(0, 4 * qh - 1)
                    blist = [bstar] + [
                        b
                        for b in range(NBT)
                        if b != bstar
                        and b * 128 < qb0
                        and (min(b + 4, NBT - 1) + 1) * 128 > qa0
                    ]
                    pv = pvps.tile([128, 512], F32, tag="pv")
                    for i, b in enumerate(blist):
                        et, q0, span = e_tiles[(si, b)]
                        ghi = min(b + 4, NBT - 1)
                        s0 = max(b * 128, qa0)
                        s1 = min((ghi + 1) * 128, qb0)
                        nc.tensor.matmul(
                            pv[:, s0 - qa0 : s1 - qa0],
                            vOnes[:, b, h, :],
                            et[:, s0 - q0 : s1 - q0],
                            start=(i == 0),
                            stop=(i == len(blist) - 1),
                        )
                    # normalize: attnT[rows_h] = O / sums
                    rec = rpool.tile([64, 512], F32, tag="rec")
                    nc.vector.reciprocal_approx_fast(out=rec, in_=pv[0:64, :])
                    nc.vector.tensor_mul(
                        attnT[rows_h, p, qa0:qb0], pv[64:128, :], rec
                    )

                return f

            return [mk(qh) for qh in range(4)]

        def zip_emit(primary, fillers):
            # emit primary thunks with fillers spliced in every 2 primaries
            fi = 0
            for i, th in enumerate(primary):
                th()
                if i % 2 == 1 and fi < len(fillers):
                    fillers[fi]()
                    fi += 1
            for th in fillers[fi:]:
                th()

        for th in k_chunks(0):
            th()
        prev_pv = None
        for p in range(NPAIR):
            kc = k_chunks(p + 1) if p < NPAIR - 1 else []
            for sub in range(2):
                si = 2 * p + sub
                fillers = []
                if prev_pv is not None:
                    fillers += prev_pv
                fillers += kc[:4] if sub == 0 else kc[4:]
                zip_emit(sc_chunks(p, sub, si), fillers)
                prev_pv = pv_chunks(p, sub, si)

        # ---- phase 4: output projection (partial: contraction over the 8
        # local heads; host sums the two head-group partials per batch),
        # interleaved with the last sub's PV groups ----
        def wo_tile(qt):
            st = stpool.tile([128, 1024], F32, tag="st")
            for nh in range(2):
                ps = mmps.tile([128, 512], F32, tag="mm")
                for c in range(NPAIR):
                    nc.tensor.matmul(
                        ps,
                        attnT[:, c, qt * 128 : qt * 128 + 128],
                        woS[:, c, nh * 512 : nh * 512 + 512],
                        start=(c == 0),
                        stop=(c == NPAIR - 1),
                    )
                _evac(st[:, nh * 512 : nh * 512 + 512], ps)
            eng = nc.sync if qt % 2 == 0 else nc.scalar
            eng.dma_start(out=out_d[qt * 128 : qt * 128 + 128, :], in_=st)

        for qh in range(4):
            prev_pv[qh]()
            for qt in range(4 * qh, 4 * qh + 4):
                wo_tile(qt)

        if debug_dumps:
            for nm, tl, sh in (
                ("d_xT", xT, [128, 4 * NCH * 512]),
                ("d_qT", qT, [128, NPAIR * T]),
                ("d_kT", kT, [128, NPAIR * T]),
                ("d_vOnes", vOnes, [128, NBT * 8 * 128]),
                ("d_attnT", attnT, [128, NPAIR * T]),
            ):
                dd = nc.dram_tensor(nm, sh, BF, kind="ExternalOutput").ap()
                nc.sync.dma_start(out=dd, in_=tl)

    nc.compile()
    return nc


def _host_inputs(x, token_positions, Wq, Wk, Wv, Wo):
    x = np.asarray(x, dtype=np.float32)
    pos = np.asarray(token_positions).astype(np.float64)
    Wq = np.asarray(Wq, np.float32)
    Wk = np.asarray(Wk, np.float32)
    Wv = np.asarray(Wv, np.float32)
    Wo = np.asarray(Wo, np.float32)

    invf = THETA ** (-np.arange(32, dtype=np.float64) / 32.0)
    ang = invf[:, None] * pos[None, :]  # [32, T]
    cos_t = np.tile(np.cos(ang), (4, 1)).astype(BF16)  # [128, T]
    sin_t = np.tile(np.sin(ang), (4, 1)).astype(BF16)
    perm = np.r_[32:64, 0:32, 96:128, 64:96]
    P = np.zeros((128, 128), np.float32)
    P[np.arange(128), perm] = 1.0
    pswapT = np.ascontiguousarray(P.T).astype(BF16)
    r = np.arange(128)[:, None]  # kv row (partition)
    c = np.arange(128)[None, :]  # q col (free)
    m_causal = (c >= r).astype(BF16)
    m_window = (c <= r).astype(BF16)
    masks = np.concatenate([m_causal, m_window], axis=1)  # [128, 256]
    pack = np.ascontiguousarray(np.concatenate([cos_t, sin_t, pswapT, masks], axis=1))
    sign = np.tile(np.repeat(np.float32([-1, 1]), 32), 2).reshape(128, 1)

    def prep_qk(W, hg):
        Ws = W[hg * 512 : (hg + 1) * 512]  # [512 n, 1024 m]
        # eo-permute within each head: [64] -> [evens(32), odds(32)]
        Wp = Ws.reshape(8, 32, 2, 1024).transpose(0, 2, 1, 3).reshape(512, 1024)
        wt = Wp.T.reshape(NCH, 128, 512).transpose(1, 0, 2)  # [128, c, n']
        return np.ascontiguousarray(wt.astype(BF16)).reshape(128, NCH * 512)

    def prep_v(W, hg):
        Ws = W[hg * 512 : (hg + 1) * 512]
        wt = Ws.T.reshape(NCH, 128, 512).transpose(1, 0, 2)
        return np.ascontiguousarray(wt.astype(BF16)).reshape(128, NCH * 512)

    def prep_o(W, hg):
        WoC = W[:, hg * 512 : (hg + 1) * 512]  # [1024 n, 512 m]
        wt = WoC.T.reshape(4, 128, 1024).transpose(1, 0, 2)  # [128, c, n]
        return np.ascontiguousarray(wt.astype(BF16)).reshape(128, 4 * 1024)

    wq_hg = [prep_qk(Wq, hg) for hg in range(2)]
    wk_hg = [prep_qk(Wk, hg) for hg in range(2)]
    wv_hg = [prep_v(Wv, hg) for hg in range(2)]
    wo_hg = [prep_o(Wo, hg) for hg in range(2)]

    xt_b = []
    for b in range(B):
        xb = x[b].astype(BF16)  # [T, D]
        xt = xb.reshape(4, 512, NCH, 128).transpose(3, 0, 2, 1)  # [128,tch,c,j]
        xt_b.append(np.ascontiguousarray(xt).reshape(128, 4 * NCH * 512))

    in_maps = []
    for core in range(8):
        b, hg = divmod(core, 2)
        in_maps.append(
            {
                "xt": xt_b[b],
                "wq": wq_hg[hg],
                "wk": wk_hg[hg],
                "wv": wv_hg[hg],
                "wo": wo_hg[hg],
                "pack": pack,
                "sign_t": sign,
            }
        )
    return in_maps


def _get_nc():
    if "nc" not in _CACHE:
        _CACHE["nc"] = _build()
    return _CACHE["nc"]


def kernel(x, token_positions, Wq, Wk, Wv, Wo, _trace=False):
    from concourse.bass_utils import run_bass_kernel_spmd

    nc = _get_nc()
    in_maps = _host_inputs(x, token_positions, Wq, Wk, Wv, Wo)
    res = run_bass_kernel_spmd(nc, in_maps, core_ids=list(range(8)), trace=_trace)
    _CACHE["last_result"] = res
    out = np.zeros((B, T, D), np.float32)
    for b in range(B):
        out[b] = res.results[2 * b]["out"] + res.results[2 * b + 1]["out"]
    return out


# revision 20
# speedup vs baseline: 1.5754x; 1.0080x over previous
# Sliding-window causal multi-head attention with RoPE for Trainium2.
#
# Problem: B=4, T=2048, D=1024, H=16 heads, d_k=64, window=512.
#   q,k,v = x @ W{q,k,v}^T (split heads), RoPE(q,k), scores = q k^T / 8 with
#   mask 0 <= i-j <= 512, softmax, out = (attn @ v) concat-heads @ Wo^T.
#
# Sharding: 8 cores = (batch b in 0..3) x (head-group of 8 heads). Each core
# runs the full T=2048 sequence for its 8 heads and produces a PARTIAL output
# projection (contraction over its 512 attn dims); the host sums the two
# head-group partials per batch. Head split avoids the K/V window-overlap
# recompute and the zero-pad softmax correction a sequence split needs.
#
# Host-side prep: x and all weights are cast to bf16 and pre-transposed into
# the exact SBUF layouts the PE consumes (m-major lhsT tiles), so the device
# does plain contiguous DMA loads only — no SWDGE casts, no xbar transposes.
# Wq/Wk rows are eo-permuted per head so RoPE's rotate-half is a 32-row group
# swap (PE permutation matmul), as in cs336 rope with (evens|odds) packing.
#
# On-chip pipeline (all matmuls bf16 with fp32 PSUM accumulation):
#   - Q^T/K^T projections produce [128 = 2 heads x (evens|odds), t] tiles;
#     RoPE via host cos/sin tables + pswap permutation matmul.
#   - scores are computed transposed, S^T[kv, q] = K Q^T, per (head, kv
#     block) over the 5-block sliding window span; exp on ACT (scale=1/8
#     folded in); boundary masks applied multiplicatively post-exp on
#     gpsimd (otherwise idle).
#   - PV uses a two-segment lhsT AP [ones | V_h] so one matmul yields the
#     softmax denominator (rows 0:64) AND O^T (rows 64:128); normalization
#     is reciprocal_approx_fast + multiply into bf16 attnT tiles.
#   - scores of sub-step s are software-pipelined against PV of s-1 and the
#     next pair's K projection so the in-order PE queue never starves while
#     ACT drains the exp chain.

import dataclasses
from contextlib import ExitStack

import numpy as np
import ml_dtypes

BF16 = ml_dtypes.bfloat16

B, T, D = 4, 2048, 1024
H, DK = 16, 64
WIN = 512
THETA = 10000.0
NBT = T // 128  # 16 t/kv blocks
NCH = D // 128  # 8 contraction chunks
NPAIR = 4  # head pairs per core

_CACHE = {}


def _pair_cols(ap2d, a, b, w):
    """From a [P, F] AP over contiguous cols, build an AP over cols
    {a..a+w} then {b..b+w} (2D free: outer count 2 step b-a)."""
    base = ap2d[:, a : a + w]
    return dataclasses.replace(base, ap=[base.ap[0], [b - a, 2], [1, w]])


def _build(debug_dumps=False):
    import concourse.bass as bass
    import concourse.bacc as bacc
    import concourse.mybir as mybir
    import concourse.tile as tile

    dt = mybir.dt
    F32, BF = dt.float32, dt.bfloat16
    AF = mybir.ActivationFunctionType
    OP = mybir.AluOpType

    nc = bacc.Bacc("TRN2", target_bir_lowering=False, debug=False, num_devices=8)

    # ---- DRAM I/O (all device inputs are host-prepped bf16 layouts) ----
    xt_in = nc.dram_tensor("xt", [128, 4 * NCH * 512], BF, kind="ExternalInput").ap()
    wq_in = nc.dram_tensor("wq", [128, NCH * 512], BF, kind="ExternalInput").ap()
    wk_in = nc.dram_tensor("wk", [128, NCH * 512], BF, kind="ExternalInput").ap()
    wv_in = nc.dram_tensor("wv", [128, NCH * 512], BF, kind="ExternalInput").ap()
    wo_in = nc.dram_tensor("wo", [128, 4 * 1024], BF, kind="ExternalInput").ap()
    # csin = per-tch [cos(512) | sin(512)]; pk = [pswap(128) | masks(256)]
    csin_in = nc.dram_tensor("csin", [128, 4 * 2 * 512], BF, kind="ExternalInput").ap()
    pk_in = nc.dram_tensor("pk", [128, 384], BF, kind="ExternalInput").ap()
    sign_in = nc.dram_tensor("sign_t", [128, 1], F32, kind="ExternalInput").ap()
    out_d = nc.dram_tensor("out", [T, D], F32, kind="ExternalOutput").ap()

    with ExitStack() as ctx:
        tc = ctx.enter_context(tile.TileContext(nc))

        big = ctx.enter_context(tc.tile_pool(name="big", bufs=1))
        ab = ctx.enter_context(tc.tile_pool(name="ab", bufs=4))
        epool = ctx.enter_context(tc.tile_pool(name="epool", bufs=24))
        rpool = ctx.enter_context(tc.tile_pool(name="rpool", bufs=2))
        stpool = ctx.enter_context(tc.tile_pool(name="stpool", bufs=2))
        # PSUM (8 banks): proj/swap/Wo 2x1 + scores 2x2 + pv 2x1
        mmps = ctx.enter_context(tc.tile_pool(name="mmps", bufs=2, space="PSUM"))
        scps = ctx.enter_context(tc.tile_pool(name="scps", bufs=2, space="PSUM"))
        pvps = ctx.enter_context(tc.tile_pool(name="pvps", bufs=2, space="PSUM"))

        # ---- persistent SBUF ----
        xT = big.tile([128, 4, NCH, 512], BF)  # [m-part, tch, chunk, t]
        qT = big.tile([128, NPAIR, T], BF)
        kT = big.tile([128, NPAIR, T], BF)
        # per (kv block, head): [ones(64) | V_h(64)] so one PV matmul yields
        # the softmax denominator (out rows 0:64) and O^T (rows 64:128)
        vOnes = big.tile([128, NBT, 8, 128], BF)
        attnT = big.tile([128, NPAIR, T], BF)
        wqS = big.tile([128, NCH, 512], BF)
        wkS = big.tile([128, NCH, 512], BF)
        wvS = big.tile([128, NCH, 512], BF)
        woS = big.tile([128, 4, 1024], BF)
        csinS = big.tile([128, 4, 2, 512], BF)
        pkS = big.tile([128, 384], BF)
        signS = big.tile([128, 1], F32)
        pswapS = pkS[:, 0:128]
        maskS = pkS[:, 128:384]

        # ---- input DMAs: every tensor split in half across the two HWDGE
        # queues, emitted in strict first-use order so the first Q/V tiles
        # can start ~4us in instead of waiting behind monolithic loads ----
        nc.vector.memset(vOnes[:, :, :, 0:64], 1.0)

        def dma2(dst_lo, src_lo, dst_hi, src_hi):
            nc.scalar.dma_start(out=dst_lo, in_=src_lo)
            nc.sync.dma_start(out=dst_hi, in_=src_hi)

        def dma_w8(dst, src):  # [128, 8, 512] weight halves
            dma2(dst[:, 0:4, :], src[:, 0:2048], dst[:, 4:8, :], src[:, 2048:4096])

        def dma_csin(tch):
            c0 = tch * 1024
            dma2(
                csinS[:, tch, 0, :], csin_in[:, c0 : c0 + 512],
                csinS[:, tch, 1, :], csin_in[:, c0 + 512 : c0 + 1024],
            )

        def dma_xt(tch):
            c0 = tch * NCH * 512
            dma2(
                xT[:, tch, 0:4, :], xt_in[:, c0 : c0 + 2048],
                xT[:, tch, 4:8, :], xt_in[:, c0 + 2048 : c0 + 4096],
            )

        dma_w8(wqS, wq_in)
        nc.scalar.dma_start(out=signS, in_=sign_in)
        nc.sync.dma_start(out=pkS, in_=pk_in)
        dma_csin(0)
        dma_xt(0)
        dma_w8(wvS, wv_in)
        dma_xt(1)
        dma_csin(1)
        dma_csin(2)
        dma_w8(wkS, wk_in)
        dma_xt(2)
        dma_xt(3)
        dma_csin(3)
        dma2(woS[:, 0:2, :], wo_in[:, 0:2048], woS[:, 2:4, :], wo_in[:, 2048:4096])

        _evac_alt = [0]

        def _evac(out, in_):
            # alternate psum evacuations between ACT and DVE queues
            _evac_alt[0] ^= 1
            if _evac_alt[0]:
                nc.scalar.copy(out=out, in_=in_)
            else:
                nc.vector.tensor_copy(out, in_)

        # ---- projection tile helpers (split so the swap matmul can be
        # queued late, after other PE work, hiding the rope DVE latency) ----
        def proj_mm_r(wS, r, tch):
            ps = mmps.tile([128, 512], F32, tag="mm")
            for c in range(NCH):
                nc.tensor.matmul(
                    ps,
                    wS[:, c, r * 128 : r * 128 + 128],
                    xT[:, tch, c, :],
                    start=(c == 0),
                    stop=(c == NCH - 1),
                )
            return ps

        def rope_pre(ps, tch, evac_eng=None):
            pb = ab.tile([128, 512], BF, tag="pb")
            if evac_eng is None:
                _evac(pb, ps)
            elif evac_eng == "v":
                nc.vector.tensor_copy(pb, ps)
            w1 = ab.tile([128, 512], BF, tag="w1")
            t2 = ab.tile([128, 512], BF, tag="t2")
            nc.vector.tensor_mul(w1, pb, csinS[:, tch, 1, :])
            nc.vector.tensor_mul(t2, pb, csinS[:, tch, 0, :])
            return w1, t2

        def rope_swap(w1, t2, dest, r, tch):
            tsl = slice(tch * 512, tch * 512 + 512)
            us = mmps.tile([128, 512], F32, tag="mm")
            nc.tensor.matmul(us, pswapS, w1, start=True, stop=True)
            # rope = swap(P*sin) * sign + P*cos
            nc.vector.scalar_tensor_tensor(
                out=dest[:, r, tsl],
                in0=us,
                scalar=signS[:, 0:1],
                in1=t2,
                op0=OP.mult,
                op1=OP.add,
            )

        def v_tile(tt):
            tch, off = tt // 4, (tt % 4) * 128
            ps = mmps.tile([128, 512], F32, tag="mm")
            for c in range(NCH):
                nc.tensor.matmul(
                    ps,
                    xT[:, tch, c, off : off + 128],
                    wvS[:, c, :],
                    start=(c == 0),
                    stop=(c == NCH - 1),
                )
            _evac(vOnes[:, tt, :, 64:128], ps)

        # ---- phase 2: Q projection interleaved with V so the PE queue has
        # V work to fill Q's rope bubbles ----
        qlist = [(r, tch) for tch in range(4) for r in range(NPAIR)]  # 16
        for i in range(16):
            r, tch = qlist[i]
            ps = proj_mm_r(wqS, r, tch)
            w1, t2 = rope_pre(ps, tch)
            v_tile(i)
            rope_swap(w1, t2, qT, r, tch)

        # ---- phase 3: K projection + attention, software-pipelined ----
        # sub-step si = 2p + sub. Scores of si interleave (in the PE queue)
        # with PV groups of si-1 and the next pair's K-projection chunks so
        # the PE keeps busy while ACT drains the per-block exp chain.
        e_tiles = {}

        def k_chunks(p):
            # 8 thunks: 4 mm chains and 4 swap finishes, swap_i after mm_i
            thunks = []
            pend = {}

            def mk_mm(tch):
                def f():
                    ps = proj_mm_r(wkS, p, tch)
                    pend[tch] = rope_pre(ps, tch, evac_eng="v")

                return f

            def mk_swap(tch):
                def f():
                    w1, t2 = pend.pop(tch)
                    rope_swap(w1, t2, kT, p, tch)

                return f

            order = [mk_mm(0), mk_mm(1), mk_swap(0), mk_mm(2), mk_swap(1),
                     mk_mm(3), mk_swap(2), mk_swap(3)]
            return order

        def sc_chunks(p, sub, si):
            rows = slice(64 * sub, 64 * sub + 64)

            def mk(b):
                def f():
                    ghi = min(b + 4, NBT - 1)
                    span = (ghi - b + 1) * 128
                    q0 = b * 128
                    sc = scps.tile([128, 640], F32, tag="sc")
                    for c0 in range(0, span, 512):
                        c1 = min(c0 + 512, span)
                        nc.tensor.matmul(
                            sc[:, c0:c1],
                            kT[rows, p, b * 128 : b * 128 + 128],
                            qT[rows, p, q0 + c0 : q0 + c1],
                            start=True,
                            stop=True,
                        )
                    et = epool.tile([128, 640], BF, tag="et")
                    nc.scalar.activation(
                        out=et[:, 0:span], in_=sc[:, 0:span], func=AF.Exp, scale=0.125
                    )
                    # boundary masks (multiplicative, post-exp) on gpsimd:
                    # causal at cols 0:128 (g=b), window at span-128 (g=b+4)
                    if b <= NBT - 5:
                        sel = _pair_cols(et[:, 0:640], 0, span - 128, 128)
                        nc.gpsimd.tensor_mul(sel, sel, maskS[:, 0:256])
                    else:
                        nc.gpsimd.tensor_mul(
                            et[:, 0:128], et[:, 0:128], maskS[:, 0:128]
                        )
                    e_tiles[(si, b)] = (et, q0, span)

                return f

            return [mk(b) for b in range(NBT)]

        def pv_chunks(p, sub, si):
            h = 2 * p + sub
            rows_h = slice(64 * sub, 64 * sub + 64)

            def mk(qh):
                def f():
                    qa0, qb0 = qh * 512, qh * 512 + 512
                    bstar = max(0, 4 * qh - 1)
                    blist = [bstar] + [
                        b
                        for b in range(NBT)
                        if b != bstar
                        and b * 128 < qb0
                        and (min(b + 4, NBT - 1) + 1) * 128 > qa0
                    ]
                    pv = pvps.tile([128, 512], F32, tag="pv")
                    for i, b in enumerate(blist):
                        et, q0, span = e_tiles[(si, b)]
                        ghi = min(b + 4, NBT - 1)
                        s0 = max(b * 128, qa0)
                        s1 = min((ghi + 1) * 128, qb0)
                        nc.tensor.matmul(
                            pv[:, s0 - qa0 : s1 - qa0],
                            vOnes[:, b, h, :],
                            et[:, s0 - q0 : s1 - q0],
                            start=(i == 0),
                            stop=(i == len(blist) - 1),
                        )
                    # normalize: attnT[rows_h] = O / sums
                    rec = rpool.tile([64, 512], F32, tag="rec")
                    nc.vector.reciprocal_approx_fast(out=rec, in_=pv[0:64, :])
                    nc.vector.tensor_mul(
                        attnT[rows_h, p, qa0:qb0], pv[64:128, :], rec
                    )

                return f

            return [mk(qh) for qh in range(4)]

        def zip_emit(primary, fillers):
            # emit primary thunks with fillers spliced in every 2 primaries
            fi = 0
            for i, th in enumerate(primary):
                th()
                if i % 2 == 1 and fi < len(fillers):
                    fillers[fi]()
                    fi += 1
            for th in fillers[fi:]:
                th()

        for th in k_chunks(0):
            th()
        prev_pv = None
        for p in range(NPAIR):
            kc = k_chunks(p + 1) if p < NPAIR - 1 else []
            for sub in range(2):
                si = 2 * p + sub
                fillers = []
                if prev_pv is not None:
                    fillers += prev_pv
                fillers += kc[:4] if sub == 0 else kc[4:]
                zip_emit(sc_chunks(p, sub, si), fillers)
                prev_pv = pv_chunks(p, sub, si)

        # ---- phase 4: output projection (partial: contraction over the 8
        # local heads; host sums the two head-group partials per batch),
        # interleaved with the last sub's PV groups ----
        def wo_tile(qt):
            st = stpool.tile([128, 1024], F32, tag="st")
            for nh in range(2):
                ps = mmps.tile([128, 512], F32, tag="mm")
                for c in range(NPAIR):
                    nc.tensor.matmul(
                        ps,
                        attnT[:, c, qt * 128 : qt * 128 + 128],
                        woS[:, c, nh * 512 : nh * 512 + 512],
                        start=(c == 0),
                        stop=(c == NPAIR - 1),
                    )
                # ACT only: DVE is busy with the last PV normalizations
                nc.scalar.copy(out=st[:, nh * 512 : nh * 512 + 512], in_=ps)
            eng = nc.sync if qt % 2 == 0 else nc.scalar
            eng.dma_start(out=out_d[qt * 128 : qt * 128 + 128, :], in_=st)

        for qh in range(4):
            prev_pv[qh]()
            for qt in range(4 * qh, 4 * qh + 4):
                wo_tile(qt)

        if debug_dumps:
            for nm, tl, sh in (
                ("d_xT", xT, [128, 4 * NCH * 512]),
                ("d_qT", qT, [128, NPAIR * T]),
                ("d_kT", kT, [128, NPAIR * T]),
                ("d_vOnes", vOnes, [128, NBT * 8 * 128]),
                ("d_attnT", attnT, [128, NPAIR * T]),
            ):
                dd = nc.dram_tensor(nm, sh, BF, kind="ExternalOutput").ap()
                nc.sync.dma_start(out=dd, in_=tl)

    nc.compile()
    return nc


def _host_inputs(x, token_positions, Wq, Wk, Wv, Wo):
    x = np.asarray(x, dtype=np.float32)
    pos = np.asarray(token_positions).astype(np.float64)
    Wq = np.asarray(Wq, np.float32)
    Wk = np.asarray(Wk, np.float32)
    Wv = np.asarray(Wv, np.float32)
    Wo = np.asarray(Wo, np.float32)

    invf = THETA ** (-np.arange(32, dtype=np.float64) / 32.0)
    ang = invf[:, None] * pos[None, :]  # [32, T]
    cos_t = np.tile(np.cos(ang), (4, 1)).astype(BF16)  # [128, T]
    sin_t = np.tile(np.sin(ang), (4, 1)).astype(BF16)
    # per-tch [cos(512) | sin(512)] chunks for just-in-time DMA
    csin = np.stack(
        [cos_t.reshape(128, 4, 512), sin_t.reshape(128, 4, 512)], axis=2
    )  # [128, 4, 2, 512]
    csin = np.ascontiguousarray(csin).reshape(128, 4096)
    perm = np.r_[32:64, 0:32, 96:128, 64:96]
    P = np.zeros((128, 128), np.float32)
    P[np.arange(128), perm] = 1.0
    pswapT = np.ascontiguousarray(P.T).astype(BF16)
    r = np.arange(128)[:, None]  # kv row (partition)
    c = np.arange(128)[None, :]  # q col (free)
    m_causal = (c >= r).astype(BF16)
    m_window = (c <= r).astype(BF16)
    pk = np.ascontiguousarray(
        np.concatenate([pswapT, m_causal, m_window], axis=1)
    )  # [128, 384]
    sign = np.tile(np.repeat(np.float32([-1, 1]), 32), 2).reshape(128, 1)

    def prep_qk(W, hg):
        Ws = W[hg * 512 : (hg + 1) * 512]  # [512 n, 1024 m]
        # eo-permute within each head: [64] -> [evens(32), odds(32)]
        Wp = Ws.reshape(8, 32, 2, 1024).transpose(0, 2, 1, 3).reshape(512, 1024)
        wt = Wp.T.reshape(NCH, 128, 512).transpose(1, 0, 2)  # [128, c, n']
        return np.ascontiguousarray(wt.astype(BF16)).reshape(128, NCH * 512)

    def prep_v(W, hg):
        Ws = W[hg * 512 : (hg + 1) * 512]
        wt = Ws.T.reshape(NCH, 128, 512).transpose(1, 0, 2)
        return np.ascontiguousarray(wt.astype(BF16)).reshape(128, NCH * 512)

    def prep_o(W, hg):
        WoC = W[:, hg * 512 : (hg + 1) * 512]  # [1024 n, 512 m]
        wt = WoC.T.reshape(4, 128, 1024).transpose(1, 0, 2)  # [128, c, n]
        return np.ascontiguousarray(wt.astype(BF16)).reshape(128, 4 * 1024)

    wq_hg = [prep_qk(Wq, hg) for hg in range(2)]
    wk_hg = [prep_qk(Wk, hg) for hg in range(2)]
    wv_hg = [prep_v(Wv, hg) for hg in range(2)]
    wo_hg = [prep_o(Wo, hg) for hg in range(2)]

    xt_b = []
    for b in range(B):
        xb = x[b].astype(BF16)  # [T, D]
        xt = xb.reshape(4, 512, NCH, 128).transpose(3, 0, 2, 1)  # [128,tch,c,j]
        xt_b.append(np.ascontiguousarray(xt).reshape(128, 4 * NCH * 512))

    in_maps = []
    for core in range(8):
        b, hg = divmod(core, 2)
        in_maps.append(
            {
                "xt": xt_b[b],
                "wq": wq_hg[hg],
                "wk": wk_hg[hg],
                "wv": wv_hg[hg],
                "wo": wo_hg[hg],
                "csin": csin,
                "pk": pk,
                "sign_t": sign,
            }
        )
    return in_maps


def _get_nc():
    if "nc" not in _CACHE:
        _CACHE["nc"] = _build()
    return _CACHE["nc"]


def kernel(x, token_positions, Wq, Wk, Wv, Wo, _trace=False):
    from concourse.bass_utils import run_bass_kernel_spmd

    nc = _get_nc()
    in_maps = _host_inputs(x, token_positions, Wq, Wk, Wv, Wo)
    res = run_bass_kernel_spmd(nc, in_maps, core_ids=list(range(8)), trace=_trace)
    _CACHE["last_result"] = res
    out = np.zeros((B, T, D), np.float32)
    for b in range(B):
        out[b] = res.results[2 * b]["out"] + res.results[2 * b + 1]["out"]
    return out
